# revision 9
# baseline (speedup 1.0000x reference)
"""ACT-RNN (adaptive computation time GRU) Trainium2 kernel, 8-core data-parallel.

Strategy:
- Shard batch (4096) across 8 cores -> 512 rows/core; replicate weights.
- Feature-on-partition layout: hT (H=1024 part x 512 batch free), so the
  recurrent matmul ghT = W_hh @ h^T needs no per-depth transposes.
- gi = inputs @ W_ih[:, :H]^T + b_ih + b_hh precomputed ONCE in exact fp32
  (inputs are constant across depth); the depth-flag column W_ih[:, H] is
  folded in as a per-partition scalar add for depths 0,1.
- Recurrent matmuls run in float32r; h state kept in exact f32 (shadow),
  rounded to float32r (double-buffered, per-group) as matmul input.  The
  halting-prob dot product and state machine run in exact f32 on DVE/ACT.
- Depth 9 computes no GRU update (its h_new is discarded by the reference).
"""

import sys

sys.path.insert(0, "/opt/trn_rl_repo")

import numpy as np

import concourse.bass as bass
import concourse.bacc as bacc
import concourse.mybir as mybir
import concourse.tile as tile
from concourse.bass_utils import run_bass_kernel_spmd

F32 = mybir.dt.float32
F32R = mybir.dt.float32r
AF = mybir.ActivationFunctionType
OP = mybir.AluOpType

N_CORES = 8
B, H = 4096, 1024
BL = B // N_CORES          # 512 batch rows per core
KT = H // 128              # 8 k tiles
JT = 3 * H // 128          # 24 j tiles (gate rows)
MAX_DEPTH = 10
THRESHOLD = 0.9


def build_nc():
    nc = bacc.Bacc("TRN2", target_bir_lowering=False, debug=False,
                   num_devices=N_CORES)

    inpT = nc.declare_dram_parameter("inpT", [KT, 128, BL], F32, isOutput=False)
    h0T = nc.declare_dram_parameter("h0T", [KT, 128, BL], F32, isOutput=False)
    whhT = nc.declare_dram_parameter("whhT", [KT, 128, 3 * H], F32, isOutput=False)
    wihT = nc.declare_dram_parameter("wihT", [JT, 128, KT * 128], F32, isOutput=False)
    wcol = nc.declare_dram_parameter("wcol", [128, JT], F32, isOutput=False)
    biht = nc.declare_dram_parameter("biht", [128, JT], F32, isOutput=False)
    bhht = nc.declare_dram_parameter("bhht", [128, JT], F32, isOutput=False)
    whalt = nc.declare_dram_parameter("whalt", [128, KT], F32, isOutput=False)
    bhalt = nc.declare_dram_parameter("bhalt", [1, 1], F32, isOutput=False)

    mergedT = nc.declare_dram_parameter("mergedT", [KT, 128, BL], F32, isOutput=True)
    hacc_o = nc.declare_dram_parameter("hacc", [1, BL], F32, isOutput=True)
    nupd_o = nc.declare_dram_parameter("nupd", [1, BL], F32, isOutput=True)

    # gi slab, grouped by k-tile: [t][:, g*BL:(g+1)*BL] = gate g of j = g*8+t
    gi_dram = nc.dram_tensor("gi_slab", [KT, 128, 3 * BL], F32)

    with tile.TileContext(nc) as tc:
        with (
            tc.tile_pool(name="const", bufs=1) as cpool,
            tc.tile_pool(name="state", bufs=1) as spool,
            tc.tile_pool(name="h32p", bufs=1) as h32p,
            tc.tile_pool(name="hrp", bufs=2) as hrp,
            tc.tile_pool(name="gipool", bufs=1) as gipool,
        ):
            ones_col = cpool.tile([128, 1], F32)      # lhsT for partition-reduce
            ones_row = cpool.tile([1, 128], F32)      # lhsT for broadcast
            nc.gpsimd.memset(ones_col[:], 1.0)
            nc.gpsimd.memset(ones_row[:], 1.0)
            wcol_t = cpool.tile([128, JT], F32)
            bsum_t = cpool.tile([128, JT], F32)
            whalt_t = cpool.tile([128, KT], F32)
            bhalt_t = cpool.tile([1, 1], F32)
            nc.sync.dma_start(out=wcol_t[:], in_=wcol[:])
            nc.sync.dma_start(out=whalt_t[:], in_=whalt[:])
            nc.sync.dma_start(out=bhalt_t[:], in_=bhalt[:])
            tmpb = cpool.tile([128, JT], F32)
            nc.sync.dma_start(out=bsum_t[:], in_=biht[:])
            nc.sync.dma_start(out=tmpb[:], in_=bhht[:])
            nc.vector.tensor_tensor(out=bsum_t[:], in0=bsum_t[:], in1=tmpb[:],
                                    op=OP.add)

            merged = [spool.tile([128, BL], F32, tag=f"mg{k}", name=f"merged{k}")
                      for k in range(KT)]
            acc = spool.tile([1, BL], F32)
            hacc = spool.tile([1, BL], F32)
            nupd = spool.tile([1, BL], F32)
            nc.gpsimd.memset(acc[:], 0.0)
            nc.gpsimd.memset(hacc[:], 0.0)
            nc.gpsimd.memset(nupd[:], 0.0)

            h32 = [h32p.tile([128, BL], F32, tag=f"h32_{k}", name=f"h32_{k}")
                   for k in range(KT)]
            h_r = [hrp.tile([128, BL], F32R, tag=f"hr{k}", name=f"hr0_{k}")
                   for k in range(KT)]

            # -------- setup: gi precompute (exact fp32) + h0 load --------
            with (
                tc.tile_pool(name="xr", bufs=1) as xrpool,
                tc.tile_pool(name="wst", bufs=2) as wstpool,
                tc.tile_pool(name="giev", bufs=2) as gievp,
                tc.tile_pool(name="gipsum", bufs=2, space="PSUM") as gipsum,
            ):
                xr = []
                for k in range(KT):
                    x = xrpool.tile([128, BL], F32, tag=f"xr{k}")
                    nc.sync.dma_start(out=x[:], in_=inpT[k])
                    xr.append(x)
                    nc.sync.dma_start(out=h32[k][:], in_=h0T[k])
                    if k % 2 == 0:
                        nc.vector.tensor_copy(h_r[k][:], h32[k][:])
                    else:
                        nc.scalar.activation(h_r[k][:], h32[k][:], AF.Copy)

                for jt in range(JT):
                    wst = wstpool.tile([128, KT * 128], F32, tag="wst")
                    nc.sync.dma_start(out=wst[:], in_=wihT[jt])
                    ps = gipsum.tile([128, BL], F32, tag="gi")
                    for k in range(KT):
                        nc.tensor.matmul(ps[:], wst[:, k * 128:(k + 1) * 128],
                                         xr[k][:],
                                         start=(k == 0), stop=(k == KT - 1))
                    ev = gievp.tile([128, BL], F32, tag="ev")
                    nc.scalar.activation(ev[:], ps[:], AF.Identity,
                                         bias=bsum_t[:, jt:jt + 1])
                    g, t = jt // KT, jt % KT
                    nc.sync.dma_start(out=gi_dram[t, :, g * BL:(g + 1) * BL],
                                      in_=ev[:])

            # ---------------- W_hh load + round ----------------
            with tc.tile_pool(name="whh", bufs=1) as whhpool:
                whh = []
                with tc.tile_pool(name="whhst", bufs=2) as whhst:
                    for k in range(KT):
                        stg = whhst.tile([128, 3 * H], F32, tag="wstg")
                        nc.sync.dma_start(out=stg[:], in_=whhT[k])
                        w = whhpool.tile([128, 3 * H], F32R, tag=f"whh{k}")
                        nc.vector.tensor_copy(w[:], stg[:])
                        whh.append(w)

                # ---------------- depth loop ----------------
                with (
                    tc.tile_pool(name="gates", bufs=1) as gpool,
                    tc.tile_pool(name="qpool", bufs=1) as qpool,
                    tc.tile_pool(name="sm", bufs=1) as smpool,
                    tc.tile_pool(name="ghpsum", bufs=2, space="PSUM") as ghpsum,
                    tc.tile_pool(name="mpsum", bufs=1, space="PSUM") as mpsum,
                ):
                    for d in range(MAX_DEPTH):
                        last = d == MAX_DEPTH - 1
                        # ---- halting head: p = sigmoid(h . w_halt + b) ----
                        q = qpool.tile([128, BL], F32, tag="q")
                        nc.vector.tensor_scalar_mul(q[:], h32[0][:],
                                                    whalt_t[:, 0:1])
                        for k in range(1, KT):
                            nc.vector.scalar_tensor_tensor(
                                out=q[:], in0=h32[k][:],
                                scalar=whalt_t[:, k:k + 1], in1=q[:],
                                op0=OP.mult, op1=OP.add)
                        plog = mpsum.tile([1, BL], F32, tag="pred")
                        nc.tensor.matmul(plog[:], ones_col[:], q[:],
                                         start=True, stop=True)
                        p = smpool.tile([1, BL], F32, tag="p")
                        nc.scalar.activation(p[:], plog[:], AF.Sigmoid,
                                             bias=bhalt_t[:])

                        # ---- halting state machine (exact f32, in-place) ----
                        t3 = smpool.tile([1, BL], F32, tag="t3")
                        if not last:
                            alive = smpool.tile([1, BL], F32, tag="t1")
                            nc.vector.tensor_scalar(alive[:], acc[:], 1.0, None,
                                                    OP.is_lt)
                            val = smpool.tile([1, BL], F32, tag="t2")
                            nc.vector.tensor_tensor(out=val[:], in0=p[:],
                                                    in1=alive[:], op=OP.mult)
                            nc.vector.tensor_tensor(out=val[:], in0=val[:],
                                                    in1=acc[:], op=OP.add)
                            nc.vector.tensor_scalar(val[:], val[:], THRESHOLD,
                                                    None, OP.is_gt)
                            nh = val
                            nc.vector.tensor_scalar(t3[:], nh[:], -1.0, 1.0,
                                                    OP.mult, OP.add)  # 1-nh
                            nc.vector.tensor_tensor(out=alive[:], in0=alive[:],
                                                    in1=t3[:], op=OP.mult)
                            alive2 = alive
                            nc.vector.tensor_tensor(out=p[:], in0=p[:],
                                                    in1=alive2[:], op=OP.mult)
                            s1 = p
                            nc.vector.tensor_tensor(out=hacc[:], in0=hacc[:],
                                                    in1=s1[:], op=OP.add)
                            nc.vector.tensor_tensor(out=nupd[:], in0=nupd[:],
                                                    in1=alive2[:], op=OP.add)
                            # rem = (1-acc)+1e-15 ; s2 = rem*nh ; stp = s1+s2
                            nc.vector.tensor_scalar(t3[:], acc[:], -1.0, 1.0,
                                                    OP.mult, OP.add)
                            nc.vector.tensor_scalar_add(t3[:], t3[:], 1e-15)
                            nc.vector.tensor_tensor(out=t3[:], in0=t3[:],
                                                    in1=nh[:], op=OP.mult)
                            nc.vector.tensor_tensor(out=t3[:], in0=s1[:],
                                                    in1=t3[:], op=OP.add)
                            stp = t3
                            nc.vector.tensor_tensor(out=acc[:], in0=acc[:],
                                                    in1=stp[:], op=OP.add)
                        else:
                            nc.vector.tensor_scalar(t3[:], acc[:], -1.0, 1.0,
                                                    OP.mult, OP.add)
                            nc.vector.tensor_scalar_add(t3[:], t3[:], 1e-15)
                            stp = t3  # new_halted forced -> step_p = remainder

                        # ---- broadcast step_p; merged += stp * h ----
                        spb_ps = mpsum.tile([128, BL], F32, tag="bcast")
                        nc.tensor.matmul(spb_ps[:], ones_row[:], stp[:],
                                         start=True, stop=True)
                        for k in range(KT):
                            if d == 0:
                                nc.vector.tensor_tensor(
                                    out=merged[k][:], in0=h32[k][:],
                                    in1=spb_ps[:], op=OP.mult)
                            else:
                                tmp = gpool.tile([128, BL], F32, tag="mtmp",
                                                 bufs=2)
                                nc.vector.tensor_tensor(
                                    out=tmp[:], in0=h32[k][:], in1=spb_ps[:],
                                    op=OP.mult)
                                nc.gpsimd.tensor_tensor(
                                    out=merged[k][:], in0=merged[k][:],
                                    in1=tmp[:], op=OP.add)

                        if last:
                            break

                        # ---- GRU: ghT = W_hh @ h^T, gates, h update ----
                        h_r_next = []
                        for t in range(KT):
                            ghs = []
                            for g in range(3):
                                j = g * KT + t
                                ps = ghpsum.tile([128, BL], F32, tag=f"gh{g}")
                                for k in range(KT):
                                    nc.tensor.matmul(
                                        ps[:],
                                        whh[k][:, j * 128:(j + 1) * 128],
                                        h_r[k][:],
                                        start=(k == 0), stop=(k == KT - 1))
                                ghs.append(ps)
                            gh_r, gh_z, gh_n = ghs
                            gi_t = gipool.tile([128, 3 * BL], F32, tag="gi")
                            nc.sync.dma_start(out=gi_t[:], in_=gi_dram[t])
                            gi_r = gi_t[:, 0:BL]
                            gi_z = gi_t[:, BL:2 * BL]
                            gi_n = gi_t[:, 2 * BL:3 * BL]
                            flag = d < 2
                            if flag:
                                nc.vector.scalar_tensor_tensor(
                                    out=gh_r[:], in0=gh_r[:],
                                    scalar=wcol_t[:, t:t + 1], in1=gi_r,
                                    op0=OP.add, op1=OP.add)
                                nc.vector.scalar_tensor_tensor(
                                    out=gh_z[:], in0=gh_z[:],
                                    scalar=wcol_t[:, KT + t:KT + t + 1],
                                    in1=gi_z, op0=OP.add, op1=OP.add)
                            else:
                                nc.vector.tensor_tensor(out=gh_r[:], in0=gh_r[:],
                                                        in1=gi_r, op=OP.add)
                                nc.vector.tensor_tensor(out=gh_z[:], in0=gh_z[:],
                                                        in1=gi_z, op=OP.add)
                            r = gpool.tile([128, BL], F32, tag="r")
                            nc.scalar.activation(r[:], gh_r[:], AF.Sigmoid)
                            z = gpool.tile([128, BL], F32, tag="z")
                            nc.scalar.activation(z[:], gh_z[:], AF.Sigmoid)
                            nc.vector.tensor_tensor(out=gh_n[:], in0=r[:],
                                                    in1=gh_n[:], op=OP.mult)
                            if flag:
                                nc.vector.scalar_tensor_tensor(
                                    out=gh_n[:], in0=gi_n,
                                    scalar=wcol_t[:, 2 * KT + t:2 * KT + t + 1],
                                    in1=gh_n[:], op0=OP.add, op1=OP.add)
                            else:
                                nc.vector.tensor_tensor(out=gh_n[:], in0=gh_n[:],
                                                        in1=gi_n, op=OP.add)
                            n = gpool.tile([128, BL], F32, tag="n")
                            nc.scalar.activation(n[:], gh_n[:], AF.Tanh)
                            # h_new = n + z*(h - n), exact f32, in place
                            tmp = gpool.tile([128, BL], F32, tag="tmp")
                            nc.vector.tensor_tensor(out=tmp[:], in0=h32[t][:],
                                                    in1=n[:], op=OP.subtract)
                            nc.vector.tensor_tensor(out=tmp[:], in0=z[:],
                                                    in1=tmp[:], op=OP.mult)
                            nc.vector.tensor_tensor(out=h32[t][:], in0=n[:],
                                                    in1=tmp[:], op=OP.add)
                            # round for next depth right away (double-buffered)
                            hr_n = hrp.tile([128, BL], F32R, tag=f"hr{t}",
                                            name=f"hr{d + 1}_{t}")
                            if t % 2 == 0:
                                nc.vector.tensor_copy(hr_n[:], h32[t][:])
                            else:
                                nc.scalar.activation(hr_n[:], h32[t][:], AF.Copy)
                            h_r_next.append(hr_n)
                        h_r = h_r_next

                    # ---- outputs ----
                    for k in range(KT):
                        nc.sync.dma_start(out=mergedT[k], in_=merged[k][:])
                    nc.sync.dma_start(out=hacc_o[:], in_=hacc[:])
                    nc.sync.dma_start(out=nupd_o[:], in_=nupd[:])

    nc.compile()
    return nc


_NC_CACHE = None


def _get_nc():
    global _NC_CACHE
    if _NC_CACHE is None:
        _NC_CACHE = build_nc()
    return _NC_CACHE


def _prep_shared(W_ih, W_hh, b_ih, b_hh, w_halt, b_halt):
    """Host-side re-layout of the replicated weights (same for all cores)."""
    W_ih = np.asarray(W_ih, np.float32)
    W_hh = np.asarray(W_hh, np.float32)
    whhT = np.ascontiguousarray(W_hh.T.reshape(KT, 128, 3 * H))
    wihT = np.ascontiguousarray(
        W_ih[:, :H].T.reshape(KT, 128, JT, 128).transpose(2, 1, 0, 3)
        .reshape(JT, 128, KT * 128))
    wcol = np.ascontiguousarray(W_ih[:, H].reshape(JT, 128).T)
    biht = np.ascontiguousarray(np.asarray(b_ih, np.float32).reshape(JT, 128).T)
    bhht = np.ascontiguousarray(np.asarray(b_hh, np.float32).reshape(JT, 128).T)
    whalt = np.ascontiguousarray(np.asarray(w_halt, np.float32).reshape(KT, 128).T)
    bhalt = np.asarray(b_halt, np.float32).reshape(1, 1)
    return dict(whhT=whhT, wihT=wihT, wcol=wcol, biht=biht, bhht=bhht,
                whalt=whalt, bhalt=bhalt)


def kernel(inputs, h0, W_ih, W_hh, b_ih, b_hh, w_halt, b_halt,
           _trace=False, _tmpdir=None):
    nc = _get_nc()
    inputs = np.asarray(inputs, np.float32)
    h0 = np.asarray(h0, np.float32)
    shared = _prep_shared(W_ih, W_hh, b_ih, b_hh, w_halt, b_halt)

    in_maps = []
    for c in range(N_CORES):
        sl = slice(c * BL, (c + 1) * BL)
        inpT = np.ascontiguousarray(inputs[sl].T.reshape(KT, 128, BL))
        h0T = np.ascontiguousarray(h0[sl].T.reshape(KT, 128, BL))
        in_maps.append(dict(inpT=inpT, h0T=h0T, **shared))

    kw = {}
    if _trace:
        kw = dict(trace=True, trace_cores=list(range(N_CORES)), tmpdir=_tmpdir)
    res = run_bass_kernel_spmd(nc, in_maps, core_ids=list(range(N_CORES)), **kw)

    merged = np.concatenate(
        [res.results[c]["mergedT"].reshape(H, BL).T for c in range(N_CORES)],
        axis=0)
    hacc = np.concatenate(
        [res.results[c]["hacc"].reshape(BL) for c in range(N_CORES)])
    nupd = np.concatenate(
        [res.results[c]["nupd"].reshape(BL) for c in range(N_CORES)])
    kernel._last_exec_time_ns = res.exec_time_ns
    return merged, merged, hacc, nupd


# revision 10
# speedup vs baseline: 1.2837x; 1.2837x over previous
"""ACT-RNN (adaptive computation time GRU) Trainium2 kernel, 8-core data-parallel.

Strategy:
- Shard batch (4096) across 8 cores -> 512 rows/core; replicate weights.
- Feature-on-partition layout: hT (H=1024 part x 512 batch free), so the
  recurrent matmul ghT = W_hh @ h^T needs no per-depth transposes.
- gi = inputs @ W_ih[:, :H]^T + b_ih + b_hh precomputed ONCE (inputs are
  constant across depth); the depth-flag column W_ih[:, H] is folded in as a
  per-partition scalar add for depths 0,1.
- Matmuls run in float32r (full PE rate); h state kept in exact f32 (shadow),
  rounded to float32r only as matmul input each depth.  Halting-prob dot
  product and the whole halting state machine run in exact f32 on DVE/ACT.
- Depth 9 computes no GRU update (its h_new is discarded by the reference).
"""

import sys

sys.path.insert(0, "/opt/trn_rl_repo")

import numpy as np

import concourse.bass as bass
import concourse.bacc as bacc
import concourse.mybir as mybir
import concourse.tile as tile
from concourse.bass_utils import run_bass_kernel_spmd

F32 = mybir.dt.float32
F32R = mybir.dt.float32r
AF = mybir.ActivationFunctionType
OP = mybir.AluOpType

N_CORES = 8
B, H = 4096, 1024
BL = B // N_CORES          # 512 batch rows per core
KT = H // 128              # 8 k tiles
JT = 3 * H // 128          # 24 j tiles (gate rows)
MAX_DEPTH = 10
THRESHOLD = 0.9


def build_nc():
    nc = bacc.Bacc("TRN2", target_bir_lowering=False, debug=False,
                   num_devices=N_CORES)

    # ---- DRAM parameters (host pre-layouts everything) ----
    inpT = nc.declare_dram_parameter("inpT", [KT, 128, BL], F32, isOutput=False)
    h0T = nc.declare_dram_parameter("h0T", [KT, 128, BL], F32, isOutput=False)
    whhT = nc.declare_dram_parameter("whhT", [KT, 128, 3 * H], F32, isOutput=False)
    wihT = nc.declare_dram_parameter("wihT", [JT, 128, KT * 128], F32, isOutput=False)
    wcol = nc.declare_dram_parameter("wcol", [128, JT], F32, isOutput=False)
    biht = nc.declare_dram_parameter("biht", [128, JT], F32, isOutput=False)
    bhht = nc.declare_dram_parameter("bhht", [128, JT], F32, isOutput=False)
    whalt = nc.declare_dram_parameter("whalt", [128, KT], F32, isOutput=False)
    bhalt = nc.declare_dram_parameter("bhalt", [1, 1], F32, isOutput=False)

    mergedT = nc.declare_dram_parameter("mergedT", [KT, 128, BL], F32, isOutput=True)
    hacc_o = nc.declare_dram_parameter("hacc", [1, BL], F32, isOutput=True)
    nupd_o = nc.declare_dram_parameter("nupd", [1, BL], F32, isOutput=True)

    # gi slab, grouped by k-tile: [t][:, g*BL:(g+1)*BL] = gates r/z/n of j=g*8+t
    gi_dram = nc.dram_tensor("gi_slab", [KT, 128, 3 * BL], F32)

    with tile.TileContext(nc) as tc:
        with (
            tc.tile_pool(name="const", bufs=1) as cpool,
            tc.tile_pool(name="state", bufs=1) as spool,
            tc.tile_pool(name="h32p", bufs=1) as h32p,
            tc.tile_pool(name="hrp", bufs=2) as hrp,
            tc.tile_pool(name="gipool", bufs=2) as gipool,
        ):
            # constants / small params
            ones_col = cpool.tile([128, 1], F32)      # lhsT for partition-reduce
            ones_row = cpool.tile([1, 128], F32)      # lhsT for broadcast
            nc.gpsimd.memset(ones_col[:], 1.0)
            nc.gpsimd.memset(ones_row[:], 1.0)
            wcol_t = cpool.tile([128, JT], F32)
            bsum_t = cpool.tile([128, JT], F32)
            whalt_t = cpool.tile([128, KT], F32)
            bhalt_t = cpool.tile([1, 1], F32)
            nc.sync.dma_start(out=wcol_t[:], in_=wcol[:])
            nc.sync.dma_start(out=whalt_t[:], in_=whalt[:])
            nc.sync.dma_start(out=bhalt_t[:], in_=bhalt[:])
            tmpb = cpool.tile([128, JT], F32)
            nc.sync.dma_start(out=bsum_t[:], in_=biht[:])
            nc.sync.dma_start(out=tmpb[:], in_=bhht[:])
            nc.vector.tensor_tensor(out=bsum_t[:], in0=bsum_t[:], in1=tmpb[:], op=OP.add)

            # persistent state
            merged = [spool.tile([128, BL], F32, tag=f"mg{k}", name=f"merged{k}")
                      for k in range(KT)]
            acc = spool.tile([1, BL], F32)
            hacc = spool.tile([1, BL], F32)
            nupd = spool.tile([1, BL], F32)
            nc.gpsimd.memset(acc[:], 0.0)
            nc.gpsimd.memset(hacc[:], 0.0)
            nc.gpsimd.memset(nupd[:], 0.0)

            # h state: exact f32 shadow + fp32r matmul copies (single-buffered)
            h32 = [h32p.tile([128, BL], F32, tag=f"h32_{k}", name=f"h32_{k}")
                   for k in range(KT)]
            h_r = [hrp.tile([128, BL], F32R, tag=f"hr{k}", name=f"hr0_{k}")
                   for k in range(KT)]

            # ---------------- setup: gi precompute + h0 load ----------------
            with (
                tc.tile_pool(name="xstage", bufs=2) as xstage,
                tc.tile_pool(name="xr", bufs=1) as xrpool,
                tc.tile_pool(name="wst", bufs=2) as wstpool,
                tc.tile_pool(name="giev", bufs=2) as gievp,
                tc.tile_pool(name="gipsum", bufs=2, space="PSUM") as gipsum,
            ):
                xr = []
                for k in range(KT):
                    x = xrpool.tile([128, BL], F32, tag=f"xr{k}")
                    nc.sync.dma_start(out=x[:], in_=inpT[k])
                    xr.append(x)
                    # h0 -> f32 shadow + fp32r
                    nc.sync.dma_start(out=h32[k][:], in_=h0T[k])
                    if k % 2 == 0:
                        nc.vector.tensor_copy(h_r[k][:], h32[k][:])
                    else:
                        nc.scalar.activation(h_r[k][:], h32[k][:], AF.Copy)

                for jt in range(JT):
                    wst = wstpool.tile([128, KT * 128], F32, tag="wst")
                    nc.sync.dma_start(out=wst[:], in_=wihT[jt])
                    ps = gipsum.tile([128, BL], F32, tag="gi")
                    for k in range(KT):
                        nc.tensor.matmul(ps[:], wst[:, k * 128:(k + 1) * 128],
                                         xr[k][:],
                                         start=(k == 0), stop=(k == KT - 1))
                    ev = gievp.tile([128, BL], F32, tag="ev")
                    nc.scalar.activation(ev[:], ps[:], AF.Identity,
                                         bias=bsum_t[:, jt:jt + 1])
                    g, t = jt // KT, jt % KT
                    nc.sync.dma_start(out=gi_dram[t, :, g * BL:(g + 1) * BL],
                                      in_=ev[:])

            # ---------------- W_hh load + round ----------------
            with tc.tile_pool(name="whh", bufs=1) as whhpool:
                whh = []
                with tc.tile_pool(name="whhst", bufs=2) as whhst:
                    for k in range(KT):
                        stg = whhst.tile([128, 3 * H], F32, tag="wstg")
                        nc.sync.dma_start(out=stg[:], in_=whhT[k])
                        w = whhpool.tile([128, 3 * H], F32R, tag=f"whh{k}")
                        nc.vector.tensor_copy(w[:], stg[:])
                        whh.append(w)

                # ---------------- depth loop ----------------
                with (
                    tc.tile_pool(name="gates", bufs=2) as gpool,
                    tc.tile_pool(name="qpool", bufs=1) as qpool,
                    tc.tile_pool(name="sm", bufs=1) as smpool,
                    tc.tile_pool(name="spb", bufs=1) as spbpool,
                    tc.tile_pool(name="ghpsum", bufs=2, space="PSUM") as ghpsum,
                    tc.tile_pool(name="mpsum", bufs=1, space="PSUM") as mpsum,
                ):
                    for d in range(MAX_DEPTH):
                        last = d == MAX_DEPTH - 1
                        # ---- halting head: p = sigmoid(h . w_halt + b) ----
                        q = qpool.tile([128, BL], F32, tag="q")
                        nc.vector.tensor_scalar_mul(q[:], h32[0][:],
                                                    whalt_t[:, 0:1])
                        for k in range(1, KT):
                            nc.vector.scalar_tensor_tensor(
                                out=q[:], in0=h32[k][:],
                                scalar=whalt_t[:, k:k + 1], in1=q[:],
                                op0=OP.mult, op1=OP.add)
                        plog = mpsum.tile([1, BL], F32, tag="pred")
                        nc.tensor.matmul(plog[:], ones_col[:], q[:],
                                         start=True, stop=True)
                        p = smpool.tile([1, BL], F32, tag="p")
                        nc.scalar.activation(p[:], plog[:], AF.Sigmoid,
                                             bias=bhalt_t[:])

                        # ---- halting state machine (exact f32, in-place) ----
                        t3 = smpool.tile([1, BL], F32, tag="t3")
                        if not last:
                            alive = smpool.tile([1, BL], F32, tag="t1")
                            nc.vector.tensor_scalar(alive[:], acc[:], 1.0, None,
                                                    OP.is_lt)
                            val = smpool.tile([1, BL], F32, tag="t2")
                            nc.vector.tensor_tensor(out=val[:], in0=p[:],
                                                    in1=alive[:], op=OP.mult)
                            nc.vector.tensor_tensor(out=val[:], in0=val[:],
                                                    in1=acc[:], op=OP.add)
                            nc.vector.tensor_scalar(val[:], val[:], THRESHOLD,
                                                    None, OP.is_gt)
                            nh = val
                            nc.vector.tensor_scalar(t3[:], nh[:], -1.0, 1.0,
                                                    OP.mult, OP.add)  # 1-nh
                            nc.vector.tensor_tensor(out=alive[:], in0=alive[:],
                                                    in1=t3[:], op=OP.mult)
                            alive2 = alive
                            nc.vector.tensor_tensor(out=p[:], in0=p[:],
                                                    in1=alive2[:], op=OP.mult)
                            s1 = p
                            nc.vector.tensor_tensor(out=hacc[:], in0=hacc[:],
                                                    in1=s1[:], op=OP.add)
                            nc.vector.tensor_tensor(out=nupd[:], in0=nupd[:],
                                                    in1=alive2[:], op=OP.add)
                            nc.vector.tensor_scalar(t3[:], acc[:], -1.0, 1.0,
                                                    OP.mult, OP.add)
                            nc.vector.tensor_scalar_add(t3[:], t3[:], 1e-15)
                            nc.vector.tensor_tensor(out=t3[:], in0=t3[:],
                                                    in1=nh[:], op=OP.mult)
                            nc.vector.tensor_tensor(out=t3[:], in0=s1[:],
                                                    in1=t3[:], op=OP.add)
                            stp = t3
                            nc.vector.tensor_tensor(out=acc[:], in0=acc[:],
                                                    in1=stp[:], op=OP.add)
                        else:
                            nc.vector.tensor_scalar(t3[:], acc[:], -1.0, 1.0,
                                                    OP.mult, OP.add)
                            nc.vector.tensor_scalar_add(t3[:], t3[:], 1e-15)
                            stp = t3  # new_halted forced -> step_p = remainder

                        # ---- broadcast step_p along partitions, merged += ----
                        spb_ps = mpsum.tile([128, BL], F32, tag="bcast")
                        nc.tensor.matmul(spb_ps[:], ones_row[:], stp[:],
                                         start=True, stop=True)
                        spb = spbpool.tile([128, BL], F32, tag="spb")
                        nc.vector.tensor_copy(spb[:], spb_ps[:])
                        for k in range(KT):
                            if d == 0:
                                nc.vector.tensor_tensor(
                                    out=merged[k][:], in0=h32[k][:],
                                    in1=spb[:], op=OP.mult)
                            else:
                                tmp = gpool.tile([128, BL], F32, tag="mtmp", bufs=2)
                                nc.vector.tensor_tensor(
                                    out=tmp[:], in0=h32[k][:], in1=spb[:],
                                    op=OP.mult)
                                nc.gpsimd.tensor_tensor(
                                    out=merged[k][:], in0=merged[k][:],
                                    in1=tmp[:], op=OP.add)

                        if last:
                            break

                        # ---- GRU: ghT = W_hh @ h^T, gates, h update ----
                        h_r_next = []
                        for t in range(KT):
                            ghs = []
                            for g in range(3):
                                j = g * KT + t
                                ps = ghpsum.tile([128, BL], F32, tag=f"gh{g}")
                                for k in range(KT):
                                    nc.tensor.matmul(
                                        ps[:],
                                        whh[k][:, j * 128:(j + 1) * 128],
                                        h_r[k][:],
                                        start=(k == 0), stop=(k == KT - 1))
                                ghs.append(ps)
                            gh_r, gh_z, gh_n = ghs
                            gi_t = gipool.tile([128, 3 * BL], F32, tag="gi")
                            nc.sync.dma_start(out=gi_t[:], in_=gi_dram[t])
                            gi_r = gi_t[:, 0:BL]
                            gi_z = gi_t[:, BL:2 * BL]
                            gi_n = gi_t[:, 2 * BL:3 * BL]
                            flag = d < 2
                            # r/z pre-activations (in-place in PSUM), then ACT
                            if flag:
                                nc.vector.scalar_tensor_tensor(
                                    out=gh_r[:], in0=gh_r[:],
                                    scalar=wcol_t[:, t:t + 1], in1=gi_r,
                                    op0=OP.add, op1=OP.add)
                                nc.vector.scalar_tensor_tensor(
                                    out=gh_z[:], in0=gh_z[:],
                                    scalar=wcol_t[:, KT + t:KT + t + 1],
                                    in1=gi_z, op0=OP.add, op1=OP.add)
                            else:
                                nc.vector.tensor_tensor(out=gh_r[:], in0=gh_r[:],
                                                        in1=gi_r, op=OP.add)
                                nc.vector.tensor_tensor(out=gh_z[:], in0=gh_z[:],
                                                        in1=gi_z, op=OP.add)
                            r = gpool.tile([128, BL], F32, tag="r")
                            nc.scalar.activation(r[:], gh_r[:], AF.Sigmoid)
                            z = gpool.tile([128, BL], F32, tag="z")
                            nc.scalar.activation(z[:], gh_z[:], AF.Sigmoid)
                            # n = tanh(gi_n (+wcol) + r*gh_n)
                            nc.vector.tensor_tensor(out=gh_n[:], in0=r[:],
                                                    in1=gh_n[:], op=OP.mult)
                            if flag:
                                nc.vector.scalar_tensor_tensor(
                                    out=gh_n[:], in0=gi_n,
                                    scalar=wcol_t[:, 2 * KT + t:2 * KT + t + 1],
                                    in1=gh_n[:], op0=OP.add, op1=OP.add)
                            else:
                                nc.vector.tensor_tensor(out=gh_n[:], in0=gh_n[:],
                                                        in1=gi_n, op=OP.add)
                            n = gpool.tile([128, BL], F32, tag="n", bufs=1)
                            nc.scalar.activation(n[:], gh_n[:], AF.Tanh)
                            # h_new = n + z*(h - n), exact f32, in place
                            tmp = gpool.tile([128, BL], F32, tag="tmp", bufs=1)
                            nc.vector.tensor_tensor(out=tmp[:], in0=h32[t][:],
                                                    in1=n[:], op=OP.subtract)
                            nc.vector.tensor_tensor(out=tmp[:], in0=z[:],
                                                    in1=tmp[:], op=OP.mult)
                            nc.vector.tensor_tensor(out=h32[t][:], in0=n[:],
                                                    in1=tmp[:], op=OP.add)
                            # round for next depth right away (double-buffered)
                            hr_n = hrp.tile([128, BL], F32R, tag=f"hr{t}",
                                            name=f"hr{d + 1}_{t}")
                            if t % 2 == 0:
                                nc.vector.tensor_copy(hr_n[:], h32[t][:])
                            else:
                                nc.scalar.activation(hr_n[:], h32[t][:], AF.Copy)
                            h_r_next.append(hr_n)
                        h_r = h_r_next

                    # ---- outputs ----
                    for k in range(KT):
                        nc.sync.dma_start(out=mergedT[k], in_=merged[k][:])
                    nc.sync.dma_start(out=hacc_o[:], in_=hacc[:])
                    nc.sync.dma_start(out=nupd_o[:], in_=nupd[:])

    nc.compile()
    return nc


_NC_CACHE = None


def _get_nc():
    global _NC_CACHE
    if _NC_CACHE is None:
        _NC_CACHE = build_nc()
    return _NC_CACHE


def _prep_shared(W_ih, W_hh, b_ih, b_hh, w_halt, b_halt):
    """Host-side re-layout of the replicated weights (same for all cores)."""
    W_ih = np.asarray(W_ih, np.float32)
    W_hh = np.asarray(W_hh, np.float32)
    whhT = np.ascontiguousarray(W_hh.T.reshape(KT, 128, 3 * H))
    wihT = np.ascontiguousarray(
        W_ih[:, :H].T.reshape(KT, 128, JT, 128).transpose(2, 1, 0, 3)
        .reshape(JT, 128, KT * 128))
    wcol = np.ascontiguousarray(W_ih[:, H].reshape(JT, 128).T)
    biht = np.ascontiguousarray(np.asarray(b_ih, np.float32).reshape(JT, 128).T)
    bhht = np.ascontiguousarray(np.asarray(b_hh, np.float32).reshape(JT, 128).T)
    whalt = np.ascontiguousarray(np.asarray(w_halt, np.float32).reshape(KT, 128).T)
    bhalt = np.asarray(b_halt, np.float32).reshape(1, 1)
    return dict(whhT=whhT, wihT=wihT, wcol=wcol, biht=biht, bhht=bhht,
                whalt=whalt, bhalt=bhalt)


def kernel(inputs, h0, W_ih, W_hh, b_ih, b_hh, w_halt, b_halt,
           _trace=False, _tmpdir=None):
    nc = _get_nc()
    inputs = np.asarray(inputs, np.float32)
    h0 = np.asarray(h0, np.float32)
    shared = _prep_shared(W_ih, W_hh, b_ih, b_hh, w_halt, b_halt)

    in_maps = []
    for c in range(N_CORES):
        sl = slice(c * BL, (c + 1) * BL)
        inpT = np.ascontiguousarray(inputs[sl].T.reshape(KT, 128, BL))
        h0T = np.ascontiguousarray(h0[sl].T.reshape(KT, 128, BL))
        in_maps.append(dict(inpT=inpT, h0T=h0T, **shared))

    kw = {}
    if _trace:
        kw = dict(trace=True, trace_cores=list(range(N_CORES)), tmpdir=_tmpdir)
    res = run_bass_kernel_spmd(nc, in_maps, core_ids=list(range(N_CORES)), **kw)

    merged = np.concatenate(
        [res.results[c]["mergedT"].reshape(H, BL).T for c in range(N_CORES)],
        axis=0)
    hacc = np.concatenate(
        [res.results[c]["hacc"].reshape(BL) for c in range(N_CORES)])
    nupd = np.concatenate(
        [res.results[c]["nupd"].reshape(BL) for c in range(N_CORES)])
    kernel._last_exec_time_ns = res.exec_time_ns
    return merged, merged, hacc, nupd
